# revision 28
# baseline (speedup 1.0000x reference)
"""Pin2PinAttraction energy kernel for 8 TRN2 NeuronCores (Bass/Tile).

E = sum_e w_e * ((x[a_e]-x[b_e])^2 + (y[a_e]-y[b_e])^2)

Sharding: edge-parallel across the 8 cores (pairs/weights split 8 ways),
per-core partial energies summed on the host (scalar all-reduce).

Division of labor (same contract as the 35us baseline this evolves from):
the axon/PJRT stack lowers vector-indirect DMA to one descriptor per SBUF
partition, making device-side gathers of 20M random pin rows orders of
magnitude slower than the roofline, so the host performs only the
index-dependent data *movement* — gathering pin xy into per-core streaming
layouts and casting to fp8 (positions fp8e3 scaled 2^-7, weights fp8e4) —
and the device computes the full energy.

The kernel is DMA-bound: the per-core SBUF-dest DMA ceiling measured in
context is ~357 GB/s (multi-stream; the documented HBM-per-NC limit), so
the layout minimizes SBUF-dest bytes (7.5 MB/core/pass) and every DMA is
a multi-bank slab with >=2 KB per-partition rows.

Device pipeline per 512-edge-column bank:
  - x-coords: fp8e3 SBUF (HWDGE slabs) -> TensorE +/-1-pattern matmul
    computes dx = xa - xb into PSUM (64 rows per matmul, two matmuls
    fill 128), ACT squares PSUM -> fp16 SBUF.
  - y-coords: fp8e3 SBUF (HWDGE slabs, split across the SP and ACT
    issue queues), DVE subtracts fp8 directly -> fp16, ACT squares.
  - weights: fp8e4 wire -> SWDGE cast DMA -> fp16 SBUF.
  - DVE: s = dx2 + dy2, wsq = s * w (fp16 x fp16 -> fp8e4).
  - TensorE: fp8 ones-matmul reduces wsq into a [1, 512] fp32 PSUM
    accumulator across all banks; the reduce-matmuls are emitted DELAY
    banks late so TensorE's in-order stream never stalls on the
    ACT/DVE chain.
Drain: PSUM accumulator -> SBUF -> DVE free-dim reduce -> [1,1] partial.
Host: sum 8 partials, undo the 2^-7 position scale (x 2^14).
"""

import numpy as np
import ml_dtypes
from contextlib import ExitStack

import concourse.bass as bass
import concourse.mybir as mybir
import concourse.tile as tile
from concourse import bacc
from concourse.bass_utils import run_bass_kernel_spmd

NUM_PINS = 2_000_000
NUM_PAIRS = 10_000_000
N_CORES = 8
P = 128
PAIRS_PER_CORE = NUM_PAIRS // N_CORES  # 1,250,000
C = -(-PAIRS_PER_CORE // P)  # 9766 edge columns per partition
E_PAD = P * C  # 1,250,048 edges incl. padding
F = 512  # bank width (one PSUM bank of fp32)
BANKS = [(k * F, min(F, C - k * F)) for k in range(-(-C // F))]  # 20 banks
POS_SCALE = 2.0 ** -7  # undone as 2^14 on the final energy
ACT_SQY_MOD = 10 ** 9  # banks where DVE (not ACT) squares dy (disabled)
REDUCE_DR = False  # DoubleRow mode for the reduce matmul
SLAB_BANKS = 5  # banks per cast-DMA slab
YA_ON_SCALAR = False  # issue ya slab DMA from ACT queue instead of SP
YB_ON_SCALAR = False  # issue yb slab DMA from ACT queue instead of SP
W_UPCONV = False  # w fp8 dest + DVE upconvert to fp16 (vs SWDGE cast)
YA_ON_GPSIMD = False  # issue ya as plain SWDGE to balance HWDGE/SWDGE paths
SLAB_BUFS = 3
CAST_DMA = True  # diagnostic: False skips y/w cast DMAs (timing only)
DMA_ONLY = False  # diagnostic: transfers only, no compute
Y_FP16_WIRE = False  # ya/yb as fp16 on the wire (HWDGE) vs fp8 cast (SWDGE)
W_FP16_WIRE = False  # w as fp16 on the wire (HWDGE) vs fp8 cast (SWDGE)

FP8E3 = ml_dtypes.float8_e3m4
FP8E4 = ml_dtypes.float8_e4m3


def build_nc(repeat=1, unroll=1):
    nc = bacc.Bacc(None, target_bir_lowering=False, debug=False)
    with tile.TileContext(nc) as tc:
        with tc.tile_pool(name="dram", bufs=1, space="DRAM") as dram:
            m = dram.tile([P, 2 * C], mybir.dt.float8e3,
                          kind="ExternalInput", name="m", uniquify=False)
            ydt = mybir.dt.float16 if Y_FP16_WIRE else mybir.dt.float8e3
            wdt = (mybir.dt.float16 if W_FP16_WIRE and not W_UPCONV
                   else mybir.dt.float8e4)
            ya = dram.tile([P, C], ydt,
                           kind="ExternalInput", name="ya", uniquify=False)
            yb = dram.tile([P, C], ydt,
                           kind="ExternalInput", name="yb", uniquify=False)
            w8 = dram.tile([P, C], wdt,
                           kind="ExternalInput", name="w8", uniquify=False)
            wpat = dram.tile([P, 64], mybir.dt.float8e3,
                             kind="ExternalInput", name="wpat", uniquify=False)
            ones = dram.tile([P, 2], mybir.dt.float8e4,
                             kind="ExternalInput", name="ones", uniquify=False)
            partial = dram.tile([1, 1], mybir.dt.float32,
                                kind="ExternalOutput", name="partial",
                                uniquify=False)
            _body(tc, m, ya, yb, w8, wpat, ones, partial, repeat, unroll)
    nc.compile()
    return nc


def _body(tc, m, ya, yb, w8, wpat, ones, partial, repeat, unroll=1):
    nc = tc.nc
    with ExitStack() as ctx:
        persist = ctx.enter_context(tc.tile_pool(name="persist", bufs=1))
        io = ctx.enter_context(tc.tile_pool(name="io", bufs=9))
        slab = ctx.enter_context(tc.tile_pool(name="slab", bufs=SLAB_BUFS))
        pd = ctx.enter_context(tc.tile_pool(name="pd", bufs=6, space="PSUM"))
        pa = ctx.enter_context(tc.tile_pool(name="pa", bufs=1, space="PSUM"))

        wp_t = persist.tile([P, 64], mybir.dt.float8e3, name="wp_t")
        on_t = persist.tile([P, 2], mybir.dt.float8e4, name="on_t")
        dr_t = persist.tile([1, F], mybir.dt.float32, name="dr_t")
        acc = pa.tile([1, F], mybir.dt.float32, name="acc")
        nc.sync.dma_start(out=wp_t[:], in_=wpat[:])
        nc.sync.dma_start(out=on_t[:], in_=ones[:])

        def one_pass(u=0):
            # y/w streams arrive in multi-bank slabs so the SWDGE cast
            # DMAs are large enough to amortize descriptor generation.
            SLAB = SLAB_BANKS * F
            slabs = [(j * SLAB, min(SLAB, C - j * SLAB))
                     for j in range(-(-C // SLAB))]
            ydt_s = mybir.dt.float16 if Y_FP16_WIRE else mybir.dt.float8e3
            slab_tiles = []
            m_slabs = []
            for j, (ss, sf) in enumerate(slabs):
                ya_s = slab.tile([P, sf], ydt_s, tag="yas",
                                 name=f"yas{u}_{j}")
                yb_s = slab.tile([P, sf], ydt_s, tag="ybs",
                                 name=f"ybs{u}_{j}")
                w_s = slab.tile([P, sf],
                                mybir.dt.float8e4 if W_UPCONV
                                else mybir.dt.float16,
                                tag="ws", name=f"ws{u}_{j}")
                if CAST_DMA:
                    if W_UPCONV:
                        w_eng = nc.sync
                    else:
                        w_eng = nc.scalar if W_FP16_WIRE else nc.gpsimd
                    if YA_ON_GPSIMD:
                        ya_eng = nc.gpsimd
                    elif YA_ON_SCALAR:
                        ya_eng = nc.scalar
                    else:
                        ya_eng = nc.sync
                    ya_eng.dma_start(out=ya_s[:], in_=ya[:, ss:ss + sf])
                    yb_eng = nc.scalar if YB_ON_SCALAR else nc.sync
                    yb_eng.dma_start(out=yb_s[:], in_=yb[:, ss:ss + sf])
                    w_eng.dma_start(out=w_s[:], in_=w8[:, ss:ss + sf])
                slab_tiles.append((ya_s, yb_s, w_s))
                m_s = slab.tile([P, 2 * sf], mybir.dt.float8e3, tag="ms",
                                name=f"ms{u}_{j}")
                nc.sync.dma_start(out=m_s[:], in_=m[:, 2 * ss:2 * ss + 2 * sf])
                m_slabs.append(m_s)

            # reduce-matmuls are emitted DELAY banks late so TensorE's
            # in-order stream never stalls on the ACT/DVE chain.
            DELAY = 6
            pend = []

            def emit_reduce(k, wsq, f):
                use_dr = REDUCE_DR and f % 32 == 0
                if use_dr:
                    f2 = f // 2
                    nc.tensor.matmul(
                        out=acc[0:1, 0:f2],
                        lhsT=on_t[:].rearrange("p (two one) -> p two one",
                                               two=2),
                        rhs=wsq[:].rearrange("p (two f2) -> p two f2", two=2),
                        start=(k == 0), stop=(k == len(BANKS) - 1),
                        perf_mode=mybir.MatmulPerfMode.DoubleRow,
                        skip_group_check=True)
                else:
                    nc.tensor.matmul(
                        out=acc[0:1, 0:f],
                        lhsT=on_t[:, 0:1], rhs=wsq[:],
                        start=(k == 0), stop=(k == len(BANKS) - 1),
                        skip_group_check=True)

            for k, (s, f) in enumerate(BANKS):
                j, off = divmod(s, SLAB)
                ya_s, yb_s, w_s = slab_tiles[j]
                sqx = io.tile([P, f], mybir.dt.float16, tag="sqx",
                              name=f"sqx{u}_{k}")
                dy = io.tile([P, f], mybir.dt.float16, tag="dy",
                             name=f"dy{u}_{k}")
                sqy = io.tile([P, f], mybir.dt.float16, tag="sqy",
                              name=f"sqy{u}_{k}")
                wsq = io.tile([P, f], mybir.dt.float8e4, tag="wsq",
                              name=f"wsq{u}_{k}")
                d_ps = pd.tile([P, f], mybir.dt.float32, tag="dps",
                               name=f"dps{u}_{k}")

                m_s = m_slabs[j]
                off2 = 2 * (s - (s // SLAB) * SLAB)
                if DMA_ONLY:
                    continue

                # dx via +/-1 pattern matmuls: rows 0:64 then 64:128
                nc.tensor.matmul(out=d_ps[0:64, :], lhsT=wp_t[:],
                                 rhs=m_s[:, off2:off2 + f], start=True,
                                 stop=True, skip_group_check=True)
                nc.tensor.matmul(out=d_ps[64:128, :], lhsT=wp_t[:],
                                 rhs=m_s[:, off2 + f:off2 + 2 * f],
                                 start=True, stop=True,
                                 skip_group_check=True)
                # sqx = dx^2 (PSUM fp32 -> SBUF fp16)
                nc.scalar.square(out=sqx[:], in_=d_ps[:])
                # dy = ya - yb
                nc.vector.tensor_tensor(out=dy[:], in0=ya_s[:, off:off + f],
                                        in1=yb_s[:, off:off + f],
                                        op=mybir.AluOpType.subtract)
                # sqy = dy^2 (balance ACT vs DVE statically)
                if k % ACT_SQY_MOD == ACT_SQY_MOD - 1:
                    nc.vector.tensor_tensor(out=sqy[:], in0=dy[:], in1=dy[:],
                                            op=mybir.AluOpType.mult)
                else:
                    nc.scalar.square(out=sqy[:], in_=dy[:])
                # s = sqx + sqy (in place over sqx), wsq = s * w -> fp8e4
                nc.vector.tensor_tensor(out=sqx[:], in0=sqx[:], in1=sqy[:],
                                        op=mybir.AluOpType.add)
                nc.vector.tensor_tensor(out=wsq[:], in0=sqx[:],
                                        in1=w_s[:, off:off + f],
                                        op=mybir.AluOpType.mult)
                pend.append((k, wsq, f))
                if len(pend) > DELAY:
                    emit_reduce(*pend.pop(0))
            while pend:
                emit_reduce(*pend.pop(0))

        if repeat == 1:
            for u in range(unroll):
                one_pass(u)
        else:
            with tc.For_i(0, repeat):
                for u in range(unroll):
                    one_pass(u)

        # drain: acc [1, F] fp32 -> SBUF -> reduce -> [1, 1]
        res = persist.tile([1, 1], mybir.dt.float32, name="res")
        if DMA_ONLY:
            nc.vector.memset(res[:], 0.0)
        else:
            nc.scalar.copy(out=dr_t[:], in_=acc[:])
            nc.vector.tensor_reduce(out=res[:], in_=dr_t[:],
                                    axis=mybir.AxisListType.XY,
                                    op=mybir.AluOpType.add)
        nc.sync.dma_start(out=partial[:], in_=res[:])


_NC_CACHE = {}


def _get_nc():
    if "nc" not in _NC_CACHE:
        _NC_CACHE["nc"] = build_nc()
    return _NC_CACHE["nc"]


def _mk_const_tiles():
    wpat = np.zeros((P, 64), dtype=FP8E3)
    for j in range(64):
        wpat[j, j] = 1.0
        wpat[64 + j, j] = -1.0
    ones = np.ones((P, 2), dtype=FP8E4)
    return wpat, ones


def _prep_in_maps(pin_pos, weights, pairs):
    pin_pos = np.asarray(pin_pos, dtype=np.float32)
    x8 = (pin_pos[:NUM_PINS] * POS_SCALE).astype(FP8E3)
    y8 = (pin_pos[NUM_PINS:] * POS_SCALE).astype(FP8E3)
    pairs = np.asarray(pairs)
    a_all = pairs[0::2]
    b_all = pairs[1::2]
    w_all = np.asarray(weights, dtype=np.float32)
    wpat, ones = _mk_const_tiles()
    in_maps = []
    for c in range(N_CORES):
        s = c * PAIRS_PER_CORE
        e = s + PAIRS_PER_CORE
        a = np.zeros(E_PAD, dtype=np.int32)
        b = np.zeros(E_PAD, dtype=np.int32)
        a[:PAIRS_PER_CORE] = a_all[s:e]
        b[:PAIRS_PER_CORE] = b_all[s:e]
        wv = np.zeros(E_PAD, dtype=np.float32)
        wv[:PAIRS_PER_CORE] = w_all[s:e]
        ag = a.reshape(P, C)
        bg = b.reshape(P, C)
        xa = x8[ag]
        xb = x8[bg]
        # m layout: per bank k cols [2s:2s+2f] = [lo_k || hi_k];
        # lo rows = (xa[0:64], xb[0:64]), hi rows = (xa[64:], xb[64:])
        m = np.empty((P, 2 * C), dtype=FP8E3)
        for k, (sk, f) in enumerate(BANKS):
            sl = slice(sk, sk + f)
            m[0:64, 2 * sk:2 * sk + f] = xa[0:64, sl]
            m[64:128, 2 * sk:2 * sk + f] = xb[0:64, sl]
            m[0:64, 2 * sk + f:2 * sk + 2 * f] = xa[64:128, sl]
            m[64:128, 2 * sk + f:2 * sk + 2 * f] = xb[64:128, sl]
        ydt = np.float16 if Y_FP16_WIRE else FP8E3
        wdt = np.float16 if W_FP16_WIRE and not W_UPCONV else FP8E4
        in_maps.append({
            "m": m,
            "ya": y8[ag].astype(ydt),
            "yb": y8[bg].astype(ydt),
            "w8": wv.reshape(P, C).astype(wdt),
            "wpat": wpat,
            "ones": ones,
        })
    return in_maps


def run_device(in_maps, trace=False, **kwargs):
    return run_bass_kernel_spmd(_get_nc(), in_maps, list(range(N_CORES)),
                                trace=trace, **kwargs)


def kernel(pin_pos, weights, pairs, pin_mask=None):
    in_maps = _prep_in_maps(pin_pos, weights, pairs)
    res = run_device(in_maps)
    total = 0.0
    for r in res.results:
        total += float(np.asarray(r["partial"], dtype=np.float64).sum())
    return np.float32(total / (POS_SCALE * POS_SCALE))


# revision 29
# speedup vs baseline: 1.3687x; 1.3687x over previous
"""Pin2PinAttraction energy kernel for 8 TRN2 NeuronCores (Bass/Tile).

E = sum_e w_e * ((x[a_e]-x[b_e])^2 + (y[a_e]-y[b_e])^2)

Sharding: edge-parallel across the 8 cores (pairs/weights split 8 ways),
per-core partial energies summed on the host (scalar all-reduce).

Division of labor (same contract as the 35us baseline this evolves from):
the axon/PJRT stack lowers vector-indirect DMA to one descriptor per SBUF
partition, making device-side gathers of 20M random pin rows orders of
magnitude slower than the roofline, so the host performs only the
index-dependent data *movement* — gathering pin xy into per-core streaming
layouts and casting to fp8 (positions fp8e3 scaled 2^-7, weights fp8e4) —
and the device computes the full energy.

The kernel is DMA-bound: the per-core SBUF-dest DMA ceiling measured in
context is ~357 GB/s (multi-stream; the documented HBM-per-NC limit), so
the layout minimizes SBUF-dest bytes (7.5 MB/core/pass) and every DMA is
a multi-bank slab with >=2 KB per-partition rows.

Device pipeline per 512-edge-column bank:
  - x-coords: fp8e3 SBUF (HWDGE slabs) -> TensorE +/-1-pattern matmul
    computes dx = xa - xb into PSUM (64 rows per matmul, two matmuls
    fill 128), ACT squares PSUM -> fp16 SBUF.
  - y-coords: fp8e3 SBUF (HWDGE slabs, split across the SP and ACT
    issue queues), DVE subtracts fp8 directly -> fp16, ACT squares.
  - weights: fp8e4 wire -> SWDGE cast DMA -> fp16 SBUF.
  - DVE: s = dx2 + dy2, wsq = s * w (fp16 x fp16 -> fp8e4).
  - TensorE: fp8 ones-matmul reduces wsq into a [1, 512] fp32 PSUM
    accumulator across all banks; the reduce-matmuls are emitted DELAY
    banks late so TensorE's in-order stream never stalls on the
    ACT/DVE chain.
Drain: PSUM accumulator -> SBUF -> DVE free-dim reduce -> [1,1] partial.
Host: sum 8 partials, undo the 2^-7 position scale (x 2^14).
"""

import numpy as np
import ml_dtypes
from contextlib import ExitStack

import concourse.bass as bass
import concourse.mybir as mybir
import concourse.tile as tile
from concourse import bacc
from concourse.bass_utils import run_bass_kernel_spmd

NUM_PINS = 2_000_000
NUM_PAIRS = 10_000_000
N_CORES = 8
P = 128
PAIRS_PER_CORE = NUM_PAIRS // N_CORES  # 1,250,000
C = -(-PAIRS_PER_CORE // P)  # 9766 edge columns per partition
E_PAD = P * C  # 1,250,048 edges incl. padding
F = 512  # bank width (one PSUM bank of fp32)
BANKS = [(k * F, min(F, C - k * F)) for k in range(-(-C // F))]  # 20 banks
# Y_TE_MOD: banks with k % Y_TE_MOD == 0 route y-coords through the
# TensorE pattern-matmul (m carries 4f cols: x_lo, x_hi, y_lo, y_hi);
# other banks subtract y on DVE from fp8 ya/yb (compacted columns).
Y_TE_MOD = 3
BANK_META = []
_m_off = 0
_y_off = 0
for _k, (_s, _f) in enumerate(BANKS):
    _y_te = (_k % Y_TE_MOD == 0)
    BANK_META.append((_s, _f, _y_te, _m_off, _y_off))
    _m_off += (4 if _y_te else 2) * _f
    if not _y_te:
        _y_off += _f
M_COLS = _m_off
YC = _y_off
POS_SCALE = 2.0 ** -7  # undone as 2^14 on the final energy
ACT_SQY_MOD = 10 ** 9  # banks where DVE (not ACT) squares dy (disabled)
REDUCE_DR = False  # DoubleRow mode for the reduce matmul
SLAB_BANKS = 5  # banks per cast-DMA slab
YA_ON_SCALAR = False  # issue ya slab DMA from ACT queue instead of SP
YB_ON_SCALAR = False  # issue yb slab DMA from ACT queue instead of SP
W_UPCONV = False  # w fp8 dest + DVE upconvert to fp16 (vs SWDGE cast)
YA_ON_GPSIMD = False  # issue ya as plain SWDGE to balance HWDGE/SWDGE paths
SLAB_BUFS = 3
CAST_DMA = True  # diagnostic: False skips y/w cast DMAs (timing only)
DMA_ONLY = False  # diagnostic: transfers only, no compute
Y_FP16_WIRE = False  # ya/yb as fp16 on the wire (HWDGE) vs fp8 cast (SWDGE)
W_FP16_WIRE = False  # w as fp16 on the wire (HWDGE) vs fp8 cast (SWDGE)

FP8E3 = ml_dtypes.float8_e3m4
FP8E4 = ml_dtypes.float8_e4m3


def build_nc(repeat=1, unroll=1):
    nc = bacc.Bacc(None, target_bir_lowering=False, debug=False)
    with tile.TileContext(nc) as tc:
        with tc.tile_pool(name="dram", bufs=1, space="DRAM") as dram:
            m = dram.tile([P, M_COLS], mybir.dt.float8e3,
                          kind="ExternalInput", name="m", uniquify=False)
            ydt = mybir.dt.float16 if Y_FP16_WIRE else mybir.dt.float8e3
            wdt = (mybir.dt.float16 if W_FP16_WIRE and not W_UPCONV
                   else mybir.dt.float8e4)
            ya = dram.tile([P, max(YC, 1)], ydt,
                           kind="ExternalInput", name="ya", uniquify=False)
            yb = dram.tile([P, max(YC, 1)], ydt,
                           kind="ExternalInput", name="yb", uniquify=False)
            w8 = dram.tile([P, C], wdt,
                           kind="ExternalInput", name="w8", uniquify=False)
            wpat = dram.tile([P, 64], mybir.dt.float8e3,
                             kind="ExternalInput", name="wpat", uniquify=False)
            ones = dram.tile([P, 2], mybir.dt.float8e4,
                             kind="ExternalInput", name="ones", uniquify=False)
            partial = dram.tile([1, 1], mybir.dt.float32,
                                kind="ExternalOutput", name="partial",
                                uniquify=False)
            _body(tc, m, ya, yb, w8, wpat, ones, partial, repeat, unroll)
    nc.compile()
    return nc


def _body(tc, m, ya, yb, w8, wpat, ones, partial, repeat, unroll=1):
    nc = tc.nc
    with ExitStack() as ctx:
        persist = ctx.enter_context(tc.tile_pool(name="persist", bufs=1))
        io = ctx.enter_context(tc.tile_pool(name="io", bufs=9))
        slab = ctx.enter_context(tc.tile_pool(name="slab", bufs=SLAB_BUFS))
        pd = ctx.enter_context(tc.tile_pool(name="pd", bufs=3, space="PSUM"))
        pa = ctx.enter_context(tc.tile_pool(name="pa", bufs=1, space="PSUM"))

        wp_t = persist.tile([P, 64], mybir.dt.float8e3, name="wp_t")
        on_t = persist.tile([P, 2], mybir.dt.float8e4, name="on_t")
        dr_t = persist.tile([1, F], mybir.dt.float32, name="dr_t")
        acc = pa.tile([1, F], mybir.dt.float32, name="acc")
        nc.sync.dma_start(out=wp_t[:], in_=wpat[:])
        nc.sync.dma_start(out=on_t[:], in_=ones[:])

        def one_pass(u=0):
            ydt_s = mybir.dt.float16 if Y_FP16_WIRE else mybir.dt.float8e3
            groups = [list(range(g, min(g + SLAB_BANKS, len(BANK_META))))
                      for g in range(0, len(BANK_META), SLAB_BANKS)]
            slab_tiles = []
            for j, grp in enumerate(groups):
                ms0 = BANK_META[grp[0]][3]
                msw = sum((4 if BANK_META[k][2] else 2) * BANK_META[k][1]
                          for k in grp)
                ys0 = BANK_META[grp[0]][4]
                ysw = sum(BANK_META[k][1] for k in grp if not BANK_META[k][2])
                ws0 = BANK_META[grp[0]][0]
                wsw = sum(BANK_META[k][1] for k in grp)
                m_s = slab.tile([P, msw], mybir.dt.float8e3, tag="ms",
                                name=f"ms{u}_{j}")
                nc.sync.dma_start(out=m_s[:], in_=m[:, ms0:ms0 + msw])
                ya_s = yb_s = None
                if ysw:
                    ya_s = slab.tile([P, ysw], ydt_s, tag="yas",
                                     name=f"yas{u}_{j}")
                    yb_s = slab.tile([P, ysw], ydt_s, tag="ybs",
                                     name=f"ybs{u}_{j}")
                    nc.sync.dma_start(out=ya_s[:], in_=ya[:, ys0:ys0 + ysw])
                    nc.sync.dma_start(out=yb_s[:], in_=yb[:, ys0:ys0 + ysw])
                w_s = slab.tile([P, wsw], mybir.dt.float16, tag="ws",
                                name=f"ws{u}_{j}")
                nc.gpsimd.dma_start(out=w_s[:], in_=w8[:, ws0:ws0 + wsw])
                slab_tiles.append((m_s, ms0, ya_s, yb_s, ys0, w_s, ws0))

            DELAY = 6
            pend = []

            def emit_reduce(k, wsq, f):
                nc.tensor.matmul(
                    out=acc[0:1, 0:f],
                    lhsT=on_t[:, 0:1], rhs=wsq[:],
                    start=(k == 0), stop=(k == len(BANKS) - 1),
                    skip_group_check=True)

            for k, (s, f, y_te, m_off, y_off) in enumerate(BANK_META):
                j = k // SLAB_BANKS
                m_s, ms0, ya_s, yb_s, ys0, w_s, ws0 = slab_tiles[j]
                mo = m_off - ms0
                wo = s - ws0
                sqx = io.tile([P, f], mybir.dt.float16, tag="sqx",
                              name=f"sqx{u}_{k}")
                sqy = io.tile([P, f], mybir.dt.float16, tag="sqy",
                              name=f"sqy{u}_{k}")
                wsq = io.tile([P, f], mybir.dt.float8e4, tag="wsq",
                              name=f"wsq{u}_{k}")
                d_ps = pd.tile([P, f], mybir.dt.float32, tag="dpx",
                               name=f"dpx{u}_{k}")
                nc.tensor.matmul(out=d_ps[0:64, :], lhsT=wp_t[:],
                                 rhs=m_s[:, mo:mo + f], start=True,
                                 stop=True, skip_group_check=True)
                nc.tensor.matmul(out=d_ps[64:128, :], lhsT=wp_t[:],
                                 rhs=m_s[:, mo + f:mo + 2 * f],
                                 start=True, stop=True,
                                 skip_group_check=True)
                nc.scalar.square(out=sqx[:], in_=d_ps[:])
                if y_te:
                    d_ps2 = pd.tile([P, f], mybir.dt.float32, tag="dpy",
                                    name=f"dpy{u}_{k}")
                    nc.tensor.matmul(out=d_ps2[0:64, :], lhsT=wp_t[:],
                                     rhs=m_s[:, mo + 2 * f:mo + 3 * f],
                                     start=True, stop=True,
                                     skip_group_check=True)
                    nc.tensor.matmul(out=d_ps2[64:128, :], lhsT=wp_t[:],
                                     rhs=m_s[:, mo + 3 * f:mo + 4 * f],
                                     start=True, stop=True,
                                     skip_group_check=True)
                    nc.scalar.square(out=sqy[:], in_=d_ps2[:])
                else:
                    yo = y_off - ys0
                    dy = io.tile([P, f], mybir.dt.float16, tag="dy",
                                 name=f"dy{u}_{k}")
                    nc.vector.tensor_tensor(out=dy[:],
                                            in0=ya_s[:, yo:yo + f],
                                            in1=yb_s[:, yo:yo + f],
                                            op=mybir.AluOpType.subtract)
                    nc.scalar.square(out=sqy[:], in_=dy[:])
                nc.vector.tensor_tensor(out=sqx[:], in0=sqx[:], in1=sqy[:],
                                        op=mybir.AluOpType.add)
                nc.vector.tensor_tensor(out=wsq[:], in0=sqx[:],
                                        in1=w_s[:, wo:wo + f],
                                        op=mybir.AluOpType.mult)
                pend.append((k, wsq, f))
                if len(pend) > DELAY:
                    emit_reduce(*pend.pop(0))
            while pend:
                emit_reduce(*pend.pop(0))

        if repeat == 1:
            for u in range(unroll):
                one_pass(u)
        else:
            with tc.For_i(0, repeat):
                for u in range(unroll):
                    one_pass(u)

        # drain: acc [1, F] fp32 -> SBUF -> reduce -> [1, 1]
        res = persist.tile([1, 1], mybir.dt.float32, name="res")
        if DMA_ONLY:
            nc.vector.memset(res[:], 0.0)
        else:
            nc.scalar.copy(out=dr_t[:], in_=acc[:])
            nc.vector.tensor_reduce(out=res[:], in_=dr_t[:],
                                    axis=mybir.AxisListType.XY,
                                    op=mybir.AluOpType.add)
        nc.sync.dma_start(out=partial[:], in_=res[:])


_NC_CACHE = {}


def _get_nc():
    if "nc" not in _NC_CACHE:
        _NC_CACHE["nc"] = build_nc()
    return _NC_CACHE["nc"]


def _mk_const_tiles():
    wpat = np.zeros((P, 64), dtype=FP8E3)
    for j in range(64):
        wpat[j, j] = 1.0
        wpat[64 + j, j] = -1.0
    ones = np.ones((P, 2), dtype=FP8E4)
    return wpat, ones


def _prep_in_maps(pin_pos, weights, pairs):
    pin_pos = np.asarray(pin_pos, dtype=np.float32)
    x8 = (pin_pos[:NUM_PINS] * POS_SCALE).astype(FP8E3)
    y8 = (pin_pos[NUM_PINS:] * POS_SCALE).astype(FP8E3)
    pairs = np.asarray(pairs)
    a_all = pairs[0::2]
    b_all = pairs[1::2]
    w_all = np.asarray(weights, dtype=np.float32)
    wpat, ones = _mk_const_tiles()
    in_maps = []
    for c in range(N_CORES):
        s = c * PAIRS_PER_CORE
        e = s + PAIRS_PER_CORE
        a = np.zeros(E_PAD, dtype=np.int32)
        b = np.zeros(E_PAD, dtype=np.int32)
        a[:PAIRS_PER_CORE] = a_all[s:e]
        b[:PAIRS_PER_CORE] = b_all[s:e]
        wv = np.zeros(E_PAD, dtype=np.float32)
        wv[:PAIRS_PER_CORE] = w_all[s:e]
        ag = a.reshape(P, C)
        bg = b.reshape(P, C)
        xa = x8[ag]
        xb = x8[bg]
        yav = y8[ag]
        ybv = y8[bg]
        m = np.empty((P, M_COLS), dtype=FP8E3)
        yac = np.empty((P, max(YC, 1)), dtype=FP8E3)
        ybc = np.empty((P, max(YC, 1)), dtype=FP8E3)
        for (sk, f, y_te, m_off, y_off) in BANK_META:
            sl = slice(sk, sk + f)
            m[0:64, m_off:m_off + f] = xa[0:64, sl]
            m[64:128, m_off:m_off + f] = xb[0:64, sl]
            m[0:64, m_off + f:m_off + 2 * f] = xa[64:128, sl]
            m[64:128, m_off + f:m_off + 2 * f] = xb[64:128, sl]
            if y_te:
                m[0:64, m_off + 2 * f:m_off + 3 * f] = yav[0:64, sl]
                m[64:128, m_off + 2 * f:m_off + 3 * f] = ybv[0:64, sl]
                m[0:64, m_off + 3 * f:m_off + 4 * f] = yav[64:128, sl]
                m[64:128, m_off + 3 * f:m_off + 4 * f] = ybv[64:128, sl]
            else:
                yac[:, y_off:y_off + f] = yav[:, sl]
                ybc[:, y_off:y_off + f] = ybv[:, sl]
        ydt = np.float16 if Y_FP16_WIRE else FP8E3
        wdt = np.float16 if W_FP16_WIRE and not W_UPCONV else FP8E4
        in_maps.append({
            "m": m,
            "ya": yac.astype(ydt),
            "yb": ybc.astype(ydt),
            "w8": wv.reshape(P, C).astype(wdt),
            "wpat": wpat,
            "ones": ones,
        })
    return in_maps


def run_device(in_maps, trace=False, **kwargs):
    return run_bass_kernel_spmd(_get_nc(), in_maps, list(range(N_CORES)),
                                trace=trace, **kwargs)


def kernel(pin_pos, weights, pairs, pin_mask=None):
    in_maps = _prep_in_maps(pin_pos, weights, pairs)
    res = run_device(in_maps)
    total = 0.0
    for r in res.results:
        total += float(np.asarray(r["partial"], dtype=np.float64).sum())
    return np.float32(total / (POS_SCALE * POS_SCALE))
